# revision 20
# baseline (speedup 1.0000x reference)
"""nn_CoPE2d Trainium2 Bass kernel. Self-contained.

Math per (b,h,half) tile (N=256, Wh=Ww=16, C=64, NPOS=288, 128 rows/tile):
  G = sigmoid(attn_logits[b,h,half])
  pos_h[n1,n2] = sum_{v'>=v, same u} G[u*16+v', n2]    (n1 = u*16+v)  -> PE matmul
  pos_w[n1,n2] = sum_{w'>=w2, same h2} G[n1, h2*16+w'] -> DVE reverse segmented scan
  pos = 16*pos_h + pos_w          (pos < 272, so the npos clamp never binds)
  L = Q @ P                        -> PE matmul  (logits_int)
  ff = floor(pos); w = pos - ff
  out = L[r,ff] + w*(L[r,ff+1] - L[r,ff])

Gather mechanism (walrus DynamicDMA facts, measured on hw by prior session):
  - descriptors are the dst AP's contiguous runs; one offset consumed per
    descriptor; offsets are consumed partition-fastest from the offset tile;
  - a 3-dim dst walks only dims[1:], so the dst is a single-partition flat
    buffer with [3,K],[1,2] runs (4B bf16 pair descriptors; hw scales the
    indirect address by the dst walk stride, so addr = 3*off elements);
  - the table is a stride-3 interleaved bf16 pair table TP2[:,3p]=L[p],
    TP2[:,3p+1]=L[p+1]-L[p] in DRAM (stride-2 gapless walks mis-gather on
    hw, and >4096 descriptors per indirect DMA crashes the device);
    gathered chunks are redistributed by one contiguous DMA per chunk;
    lerp = v0 + w*v1.
Sharding: data-parallel over B (8 b-values per core), ONE launch per core
with all 256 (b,h,half) tiles python-unrolled (walrus compile is fast).
The pair-table store and redistributes ride the ACT HWDGE ring so the Pool
SWDGE queue carries only the gathers (the bottleneck: ~13us gen + ~14us
SDMA transfer per tile; DVE ~4us, ACT ~3us, PE ~2us hide underneath).
"""
import sys
from contextlib import ExitStack

import numpy as np

sys.path.insert(0, "/opt/trn_rl_repo")

import concourse.bass as bass
import concourse.mybir as mybir
import concourse.tile as tile
from concourse import bacc
from concourse.bass import IndirectOffsetOnAxis
from concourse.bass_utils import run_bass_kernel_spmd

F32 = mybir.dt.float32
BF16 = mybir.dt.bfloat16
I32 = mybir.dt.int32
AF = mybir.ActivationFunctionType
ALU = mybir.AluOpType

B, NH, N, C, NPOS, SEG = 64, 16, 256, 64, 288, 16
NCORES = 8
BPC = B // NCORES  # b per core
R = 16             # rows per gather chunk (8 chunks per tile)

import os as _os
K_FLAT = _os.environ.get("K_FLAT", "two1p")    # "pair2" | "two1p"
K_RING = _os.environ.get("K_RING", "scalar")   # "scalar" | "base"
R = int(_os.environ.get("K_R", str(R)))        # rows per gather chunk
NCHUNK = 128 // R
K_DT = _os.environ.get("K_DT", "bf16")         # gather-path dtype


def _m16_np():
    k = np.arange(128)
    m = np.arange(128)
    M = ((k[:, None] // SEG == m[None, :] // SEG)
         & (k[:, None] % SEG >= m[None, :] % SEG))
    return M.astype(np.float32)


def _maskr_np():
    t = np.arange(N)
    return np.broadcast_to((t % SEG != 0).astype(np.float32), (128, N)).copy()


def _rowbase_np():
    return (np.arange(128, dtype=np.int64) * NPOS).astype(np.float32).reshape(128, 1)


def _build(nc, bpc=BPC, debug=False):
    GDT = BF16 if K_DT == "bf16" else F32
    A_d = nc.dram_tensor("A", [bpc, NH, N, N], F32, kind="ExternalInput")
    Q_d = nc.dram_tensor("Q", [bpc, NH, N, C], F32, kind="ExternalInput")
    P_d = nc.dram_tensor("P", [C, NPOS], F32, kind="ExternalInput")
    out_d = nc.dram_tensor("out", [bpc, NH, N, N], F32, kind="ExternalOutput")
    if debug:
        dbg_pos = nc.dram_tensor("dbg_pos", [128, N], F32, kind="ExternalOutput")
        dbg_off = nc.dram_tensor("dbg_off", [128, N], I32, kind="ExternalOutput")
        dbg_gath = nc.dram_tensor("dbg_gath", [128, 3 * N], BF16 if _os.environ.get("K_DT", "bf16") == "bf16" else F32,
                                  kind="ExternalOutput")
        dbg_w = nc.dram_tensor("dbg_w", [128, N], F32, kind="ExternalOutput")

    with tile.TileContext(nc) as tc, ExitStack() as ctx:
        sb = ctx.enter_context(tc.tile_pool(name="sb", bufs=2))
        flats = ctx.enter_context(tc.tile_pool(name="flats", bufs=1))
        const = ctx.enter_context(tc.tile_pool(name="const", bufs=1))
        psum = ctx.enter_context(tc.tile_pool(name="psum", bufs=2, space="PSUM"))
        psum2 = ctx.enter_context(tc.tile_pool(name="psum2", bufs=1, space="PSUM"))
        dra = ctx.enter_context(tc.tile_pool(name="dra", bufs=1, space="DRAM"))

        # constants
        M16_ld = const.tile([128, 128], F32, tag="M16ld")
        nc.sync.dma_start(M16_ld[:], nc.inline_tensor(_m16_np(), name="M16c")[:, :])
        M16 = const.tile([128, 128], F32, tag="M16")
        nc.vector.tensor_copy(M16[:], M16_ld[:])
        maskR = const.tile([128, N], F32, tag="maskR")
        nc.sync.dma_start(maskR[:], nc.inline_tensor(_maskr_np(), name="maskRc")[:, :])
        rowbase = const.tile([128, 1], F32, tag="rowbase")
        nc.sync.dma_start(rowbase[:], nc.inline_tensor(_rowbase_np(), name="rbc")[:, :])
        eye_ld = const.tile([128, 128], F32, tag="eyeld")
        nc.sync.dma_start(eye_ld[:], nc.inline_tensor(np.eye(128, dtype=np.float32),
                                                      name="eyec")[:, :])
        eye = const.tile([128, 128], F32, tag="eye")
        nc.vector.tensor_copy(eye[:], eye_ld[:])
        P_ld = const.tile([C, NPOS], F32, tag="Pld")
        nc.sync.dma_start(P_ld[:], P_d[:, :])
        P_sb = const.tile([C, NPOS], F32, tag="P")
        nc.vector.tensor_copy(P_sb[:], P_ld[:])

        for bi in range(bpc):
            for h in range(NH):
                for half in range(2):
                    r0 = half * 128
                    t_idx = (bi * NH + h) * 2 + half

                    A_t = sb.tile([128, N], F32, tag="A")
                    nc.sync.dma_start(A_t[:], A_d[bi, h, r0:r0 + 128, :])
                    q_t = sb.tile([128, C], F32, tag="q")
                    nc.sync.dma_start(q_t[:], Q_d[bi, h, r0:r0 + 128, :])

                    G = sb.tile([128, N], F32, tag="G")
                    nc.scalar.activation(G[:], A_t[:], AF.Sigmoid)

                    # pos_h via masked-cumsum matmul, pos_w via DVE scan
                    psum_h = psum.tile([128, N], F32, tag="ph")
                    nc.tensor.matmul(psum_h[:], M16[:], G[:], start=True, stop=True)
                    ph_sb = sb.tile([128, N], F32, tag="ph_sb")
                    nc.scalar.copy(ph_sb[:], psum_h[:])
                    posw = sb.tile([128, N], F32, tag="pw")
                    nc.vector.tensor_tensor_scan(posw[:, ::-1], maskR[:], G[:, ::-1],
                                                 0.0, ALU.mult, ALU.add)
                    pos = sb.tile([128, N], F32, tag="pos")
                    nc.vector.scalar_tensor_tensor(pos[:], ph_sb[:], 16.0, posw[:],
                                                   ALU.mult, ALU.add)

                    # ff = floor(pos) robustly under either f32->i32 convert
                    # rounding mode (trunc or round-to-nearest)
                    fi = sb.tile([128, N], I32, tag="fi")
                    nc.vector.tensor_scalar(fi[:], pos[:], 0.0, None, ALU.add)
                    ff0 = sb.tile([128, N], F32, tag="ff0")
                    nc.vector.tensor_copy(ff0[:], fi[:])
                    gtm = sb.tile([128, N], F32, tag="gtm")
                    nc.vector.tensor_tensor(gtm[:], ff0[:], pos[:], ALU.is_gt)
                    ff = sb.tile([128, N], F32, tag="ff")
                    nc.vector.tensor_tensor(ff[:], ff0[:], gtm[:], ALU.subtract)
                    w = sb.tile([128, N], F32, tag="w")
                    nc.vector.scalar_tensor_tensor(w[:], ff[:], -1.0, pos[:],
                                                   ALU.mult, ALU.add)

                    # transpose q on device: qth[c, m] = q_t[m, c]
                    psum_qt = psum.tile([C, 128], F32, tag="pqt")
                    nc.tensor.transpose(psum_qt[:], q_t[:], eye[:])
                    qth = sb.tile([C, 128], F32, tag="qth")
                    nc.scalar.copy(qth[:], psum_qt[:])

                    psum_L = psum.tile([128, NPOS], F32, tag="pl")
                    nc.tensor.matmul(psum_L[:], qth[:], P_sb[:], start=True,
                                     stop=True)
                    Lsb = sb.tile([128, NPOS], F32, tag="Lsb")
                    nc.scalar.copy(Lsb[:], psum_L[:])

                    # stride-3 pair table (the stride-2 gapless layout mis-
                    # gathers on hw): TP2[:,3p] = L[p], TP2[:,3p+1] = D[p]
                    TP2 = sb.tile([128, 3 * NPOS], GDT, tag="TP2")
                    t0 = TP2[:]
                    nc.vector.tensor_copy(
                        bass.AP(t0.tensor, t0.offset, [t0.ap[0], [3, NPOS]]),
                        Lsb[:, 0:NPOS])
                    nc.vector.scalar_tensor_tensor(
                        bass.AP(t0.tensor, t0.offset + 1, [t0.ap[0], [3, NPOS - 1]]),
                        Lsb[:, 0:NPOS - 1], -1.0, Lsb[:, 1:NPOS],
                        ALU.mult, ALU.add)
                    # unread lanes (3p+2 and the last D slot); init for sim
                    nc.vector.memset(
                        bass.AP(t0.tensor, t0.offset + 2, [t0.ap[0], [3, NPOS]]), 0)
                    nc.vector.memset(
                        bass.AP(t0.tensor, t0.offset + 3 * (NPOS - 1) + 1,
                                [t0.ap[0], [1, 1]]), 0)
                    TD = dra.tile([128, 3 * NPOS], GDT, tag="TD")
                    td_ap = TD[:]
                    assert td_ap.offset == 0, "pair table must sit at offset 0"
                    if K_RING == "scalar":
                        nc.scalar.dma_start(td_ap, TP2[:])
                    else:
                        nc.gpsimd.dma_start(td_ap, TP2[:])

                    # transposed offsets: off[q, 2a+b] = ff[a, b*128+q] + a*NPOS
                    offf = sb.tile([128, N], F32, tag="offf")
                    nc.vector.tensor_scalar(offf[:], ff[:], rowbase[:], None, ALU.add)
                    ptA = psum2.tile([128, 128], F32, tag="tA")
                    nc.tensor.transpose(ptA[:], offf[:, 0:128], eye[:])
                    ptB = psum2.tile([128, 128], F32, tag="tB")
                    nc.tensor.transpose(ptB[:], offf[:, 128:256], eye[:])
                    off = sb.tile([128, N], I32, tag="off")
                    oap = off[:]
                    nc.vector.tensor_copy(
                        bass.AP(oap.tensor, oap.offset, [oap.ap[0], [2, 128]]),
                        ptA[:])
                    nc.vector.tensor_copy(
                        bass.AP(oap.tensor, oap.offset + 1, [oap.ap[0], [2, 128]]),
                        ptB[:])

                    # pair-gather: 2 chunks of R=64 rows; src AP shaped as
                    # pairs so sim coef(=2) matches hw dst-walk-stride scaling
                    gath = sb.tile([128, 3 * N], GDT, tag="gath")
                    src = bass.AP(td_ap.tensor, 0, [[3, 128 * NPOS], [1, 3]])
                    for c in range(NCHUNK):
                        flat = flats.tile([1, 3 * R * N], GDT,
                                          tag=f"flat{c % 2}")
                        fap = flat[:]
                        fsrc = flat[:]
                        pair_dst = bass.AP(fap.tensor, fap.offset,
                                           [fap.ap[0], [3, R * N], [1, 2]])
                        off_sl = off[:, 2 * R * c: 2 * R * c + 2 * R]
                        nc.gpsimd.indirect_dma_start(
                            pair_dst, None, src,
                            IndirectOffsetOnAxis(ap=off_sl, axis=0))
                        if K_RING == "scalar":
                            nc.scalar.dma_start(gath[R * c: R * c + R, :], fsrc)
                        else:
                            nc.sync.dma_start(gath[R * c: R * c + R, :], fsrc)
                    dst = gath[:]
                    v0 = bass.AP(dst.tensor, dst.offset, [dst.ap[0], [3, N]])
                    v1 = bass.AP(dst.tensor, dst.offset + 1, [dst.ap[0], [3, N]])

                    if debug and t_idx == 0:
                        nc.sync.dma_start(dbg_pos[:, :], pos[:])
                        nc.sync.dma_start(dbg_off[:, :], off[:])
                        nc.sync.dma_start(dbg_gath[:, :], gath[:])
                        nc.sync.dma_start(dbg_w[:, :], w[:])

                    # out = v0 + w * v1
                    t1 = sb.tile([128, N], F32, tag="t1")
                    nc.vector.tensor_tensor(t1[:], w[:], v1, ALU.mult)
                    res = sb.tile([128, N], F32, tag="res")
                    nc.vector.tensor_tensor(res[:], t1[:], v0, ALU.add)
                    nc.sync.dma_start(out_d[bi, h, r0:r0 + 128, :], res[:])
    nc.compile()
    return nc


_NC_CACHE = {}


def kernel(query, attn_logits, pos_emb, Wh, Ww, npos_max):
    query = np.asarray(query, dtype=np.float32)
    attn_logits = np.asarray(attn_logits, dtype=np.float32)
    pos_emb = np.asarray(pos_emb, dtype=np.float32)

    if "nc" not in _NC_CACHE:
        _NC_CACHE["nc"] = _build(
            bacc.Bacc("TRN2", target_bir_lowering=False, num_devices=NCORES))
    nc = _NC_CACHE["nc"]

    in_maps = []
    for c in range(NCORES):
        sl = slice(c * BPC, (c + 1) * BPC)
        in_maps.append({
            "A": attn_logits[sl],
            "Q": query[sl],
            "P": pos_emb,
        })
    res = run_bass_kernel_spmd(nc, in_maps, core_ids=list(range(NCORES)))
    out = np.concatenate([res.results[c]["out"] for c in range(NCORES)], axis=0)
    return out


# revision 21
# speedup vs baseline: 2.4127x; 2.4127x over previous
"""nn_CoPE2d Trainium2 Bass kernel. Self-contained.

Math per (b,h,half) tile (N=256, Wh=Ww=16, C=64, NPOS=288, 128 rows/tile):
  G = sigmoid(attn_logits[b,h,half])
  pos_h[n1,n2] = sum_{v'>=v, same u} G[u*16+v', n2]    (n1 = u*16+v)  -> PE matmul
  pos_w[n1,n2] = sum_{w'>=w2, same h2} G[n1, h2*16+w'] -> DVE reverse segmented scan
  pos = 16*pos_h + pos_w          (pos < 272, so the npos clamp never binds)
  L = Q @ P                        -> PE matmul  (logits_int)
  ff = floor(pos); w = pos - ff
  out = L[r,ff] + w*(L[r,ff+1] - L[r,ff])

Gather mechanism (walrus DynamicDMA facts, measured on hw by prior session):
  - descriptors are the dst AP's contiguous runs; one offset consumed per
    descriptor; offsets are consumed partition-fastest from the offset tile;
  - a 3-dim dst walks only dims[1:], so the dst is a single-partition flat
    buffer with [3,K],[1,2] runs (4B bf16 pair descriptors; hw scales the
    indirect address by the dst walk stride, so addr = 3*off elements);
  - the table is a stride-3 interleaved bf16 pair table TP2[:,3p]=L[p],
    TP2[:,3p+1]=L[p+1]-L[p] in DRAM (stride-2 gapless walks mis-gather on
    hw, and >4096 descriptors per indirect DMA crashes the device);
    gathered chunks are redistributed by one contiguous DMA per chunk;
    lerp = v0 + w*v1.
Sharding: data-parallel over B (8 b-values per core), ONE launch per core
with all 256 (b,h,half) tiles python-unrolled (walrus compile is fast).
The pair-table store and redistributes ride the ACT HWDGE ring so the Pool
SWDGE queue carries only the gathers (the bottleneck: ~13us gen + ~14us
SDMA transfer per tile; DVE ~4us, ACT ~3us, PE ~2us hide underneath).
"""
import sys
from contextlib import ExitStack

import numpy as np

sys.path.insert(0, "/opt/trn_rl_repo")

import concourse.bass as bass
import concourse.mybir as mybir
import concourse.tile as tile
from concourse import bacc
from concourse.bass import IndirectOffsetOnAxis
from concourse.bass_utils import run_bass_kernel_spmd

F32 = mybir.dt.float32
F16 = mybir.dt.float16
BF16 = mybir.dt.bfloat16
I32 = mybir.dt.int32
AF = mybir.ActivationFunctionType
ALU = mybir.AluOpType

B, NH, N, C, NPOS, SEG = 64, 16, 256, 64, 288, 16
NCORES = 8
BPC = B // NCORES  # b per core
R = 16             # rows per gather chunk (8 chunks per tile)

import os as _os
K_FLAT = _os.environ.get("K_FLAT", "two1p")    # "pair2" | "two1p"
K_RING = _os.environ.get("K_RING", "scalar")   # "scalar" | "base"
R = int(_os.environ.get("K_R", str(R)))        # rows per gather chunk
NCHUNK = 128 // R
K_DT = _os.environ.get("K_DT", "bf16")         # gather-path dtype


def _m16_np():
    k = np.arange(128)
    m = np.arange(128)
    M = ((k[:, None] // SEG == m[None, :] // SEG)
         & (k[:, None] % SEG >= m[None, :] % SEG))
    return M.astype(np.float32)


def _maskr_np():
    t = np.arange(N)
    return np.broadcast_to((t % SEG != 0).astype(np.float32), (128, N)).copy()


def _rowbase_np():
    return (np.arange(128, dtype=np.int64) * NPOS).astype(np.float32).reshape(128, 1)


def _build(nc, bpc=BPC, debug=False):
    GDT = BF16 if K_DT == "bf16" else F32
    A_d = nc.dram_tensor("A", [bpc, NH, N, N], F16, kind="ExternalInput")
    Q_d = nc.dram_tensor("Q", [bpc, NH, N, C], F16, kind="ExternalInput")
    P_d = nc.dram_tensor("P", [C, NPOS], F16, kind="ExternalInput")
    out_d = nc.dram_tensor("out", [bpc, NH, N, N], BF16, kind="ExternalOutput")
    if debug:
        dbg_pos = nc.dram_tensor("dbg_pos", [128, N], F32, kind="ExternalOutput")
        dbg_off = nc.dram_tensor("dbg_off", [128, N], I32, kind="ExternalOutput")
        dbg_gath = nc.dram_tensor("dbg_gath", [128, 3 * N], BF16 if _os.environ.get("K_DT", "bf16") == "bf16" else F32,
                                  kind="ExternalOutput")
        dbg_w = nc.dram_tensor("dbg_w", [128, N], F32, kind="ExternalOutput")

    with tile.TileContext(nc) as tc, ExitStack() as ctx:
        sb = ctx.enter_context(tc.tile_pool(name="sb", bufs=2))
        flats = ctx.enter_context(tc.tile_pool(name="flats", bufs=1))
        const = ctx.enter_context(tc.tile_pool(name="const", bufs=1))
        psum = ctx.enter_context(tc.tile_pool(name="psum", bufs=2, space="PSUM"))
        psum2 = ctx.enter_context(tc.tile_pool(name="psum2", bufs=1, space="PSUM"))
        dra = ctx.enter_context(tc.tile_pool(name="dra", bufs=1, space="DRAM"))

        # constants
        M16_ld = const.tile([128, 128], F32, tag="M16ld")
        nc.sync.dma_start(M16_ld[:], nc.inline_tensor(_m16_np(), name="M16c")[:, :])
        M16 = const.tile([128, 128], F32, tag="M16")
        nc.vector.tensor_copy(M16[:], M16_ld[:])
        maskR = const.tile([128, N], F32, tag="maskR")
        nc.sync.dma_start(maskR[:], nc.inline_tensor(_maskr_np(), name="maskRc")[:, :])
        rowbase = const.tile([128, 1], F32, tag="rowbase")
        nc.sync.dma_start(rowbase[:], nc.inline_tensor(_rowbase_np(), name="rbc")[:, :])
        eye_ld = const.tile([128, 128], F32, tag="eyeld")
        nc.sync.dma_start(eye_ld[:], nc.inline_tensor(np.eye(128, dtype=np.float32),
                                                      name="eyec")[:, :])
        eye = const.tile([128, 128], F32, tag="eye")
        nc.vector.tensor_copy(eye[:], eye_ld[:])
        eye16_ld = const.tile([128, 128], F16, tag="eye16ld")
        nc.sync.dma_start(eye16_ld[:],
                          nc.inline_tensor(np.eye(128, dtype=np.float16),
                                           name="eye16c")[:, :])
        eye16 = const.tile([128, 128], F16, tag="eye16")
        nc.vector.tensor_copy(eye16[:], eye16_ld[:])
        P_ld = const.tile([C, NPOS], F16, tag="Pld")
        nc.sync.dma_start(P_ld[:], P_d[:, :])
        P_sb = const.tile([C, NPOS], F16, tag="P")
        nc.vector.tensor_copy(P_sb[:], P_ld[:])

        for bi in range(bpc):
            for h in range(NH):
                for half in range(2):
                    r0 = half * 128
                    t_idx = (bi * NH + h) * 2 + half

                    A_t = sb.tile([128, N], F16, tag="A")
                    nc.sync.dma_start(A_t[:], A_d[bi, h, r0:r0 + 128, :])
                    q_t = sb.tile([128, C], F16, tag="q")
                    nc.sync.dma_start(q_t[:], Q_d[bi, h, r0:r0 + 128, :])

                    G = sb.tile([128, N], F32, tag="G")
                    nc.scalar.activation(G[:], A_t[:], AF.Sigmoid)

                    # pos_h via masked-cumsum matmul, pos_w via DVE scan
                    psum_h = psum.tile([128, N], F32, tag="ph")
                    nc.tensor.matmul(psum_h[:], M16[:], G[:], start=True, stop=True)
                    ph_sb = sb.tile([128, N], F32, tag="ph_sb")
                    nc.scalar.copy(ph_sb[:], psum_h[:])
                    posw = sb.tile([128, N], F32, tag="pw")
                    nc.vector.tensor_tensor_scan(posw[:, ::-1], maskR[:], G[:, ::-1],
                                                 0.0, ALU.mult, ALU.add)
                    pos = sb.tile([128, N], F32, tag="pos")
                    nc.vector.scalar_tensor_tensor(pos[:], ph_sb[:], 16.0, posw[:],
                                                   ALU.mult, ALU.add)

                    # ff = floor(pos) robustly under either f32->i32 convert
                    # rounding mode (trunc or round-to-nearest)
                    fi = sb.tile([128, N], I32, tag="fi")
                    nc.vector.tensor_scalar(fi[:], pos[:], 0.0, None, ALU.add)
                    ff0 = sb.tile([128, N], F32, tag="ff0")
                    nc.vector.tensor_copy(ff0[:], fi[:])
                    gtm = sb.tile([128, N], F32, tag="gtm")
                    nc.vector.tensor_tensor(gtm[:], ff0[:], pos[:], ALU.is_gt)
                    ff = sb.tile([128, N], F32, tag="ff")
                    nc.vector.tensor_tensor(ff[:], ff0[:], gtm[:], ALU.subtract)
                    w = sb.tile([128, N], F32, tag="w")
                    nc.vector.scalar_tensor_tensor(w[:], ff[:], -1.0, pos[:],
                                                   ALU.mult, ALU.add)

                    # transpose q on device: qth[c, m] = q_t[m, c]
                    psum_qt = psum.tile([C, 128], F16, tag="pqt")
                    nc.tensor.transpose(psum_qt[:], q_t[:], eye16[:])
                    qth = sb.tile([C, 128], F16, tag="qth")
                    nc.scalar.copy(qth[:], psum_qt[:])

                    psum_L = psum.tile([128, NPOS], F32, tag="pl")
                    nc.tensor.matmul(psum_L[:], qth[:], P_sb[:], start=True,
                                     stop=True)
                    Lsb = sb.tile([128, NPOS], F32, tag="Lsb")
                    nc.scalar.copy(Lsb[:], psum_L[:])

                    # stride-3 pair table (the stride-2 gapless layout mis-
                    # gathers on hw): TP2[:,3p] = L[p], TP2[:,3p+1] = D[p]
                    TP2 = sb.tile([128, 3 * NPOS], GDT, tag="TP2")
                    t0 = TP2[:]
                    nc.vector.tensor_copy(
                        bass.AP(t0.tensor, t0.offset, [t0.ap[0], [3, NPOS]]),
                        Lsb[:, 0:NPOS])
                    nc.vector.scalar_tensor_tensor(
                        bass.AP(t0.tensor, t0.offset + 1, [t0.ap[0], [3, NPOS - 1]]),
                        Lsb[:, 0:NPOS - 1], -1.0, Lsb[:, 1:NPOS],
                        ALU.mult, ALU.add)
                    # unread lanes (3p+2 and the last D slot); init for sim
                    nc.vector.memset(
                        bass.AP(t0.tensor, t0.offset + 2, [t0.ap[0], [3, NPOS]]), 0)
                    nc.vector.memset(
                        bass.AP(t0.tensor, t0.offset + 3 * (NPOS - 1) + 1,
                                [t0.ap[0], [1, 1]]), 0)
                    TD = dra.tile([128, 3 * NPOS], GDT, tag="TD")
                    td_ap = TD[:]
                    assert td_ap.offset == 0, "pair table must sit at offset 0"
                    if K_RING == "scalar":
                        nc.scalar.dma_start(td_ap, TP2[:])
                    else:
                        nc.gpsimd.dma_start(td_ap, TP2[:])

                    # transposed offsets: off[q, 2a+b] = ff[a, b*128+q] + a*NPOS
                    offf = sb.tile([128, N], F32, tag="offf")
                    nc.vector.tensor_scalar(offf[:], ff[:], rowbase[:], None, ALU.add)
                    ptA = psum2.tile([128, 128], F32, tag="tA")
                    nc.tensor.transpose(ptA[:], offf[:, 0:128], eye[:])
                    ptB = psum2.tile([128, 128], F32, tag="tB")
                    nc.tensor.transpose(ptB[:], offf[:, 128:256], eye[:])
                    off = sb.tile([128, N], I32, tag="off")
                    oap = off[:]
                    nc.vector.tensor_copy(
                        bass.AP(oap.tensor, oap.offset, [oap.ap[0], [2, 128]]),
                        ptA[:])
                    nc.vector.tensor_copy(
                        bass.AP(oap.tensor, oap.offset + 1, [oap.ap[0], [2, 128]]),
                        ptB[:])

                    # pair-gather: 2 chunks of R=64 rows; src AP shaped as
                    # pairs so sim coef(=2) matches hw dst-walk-stride scaling
                    gath = sb.tile([128, 3 * N], GDT, tag="gath")
                    src = bass.AP(td_ap.tensor, 0, [[3, 128 * NPOS], [1, 3]])
                    for c in range(NCHUNK):
                        flat = flats.tile([1, 3 * R * N], GDT,
                                          tag=f"flat{c % 2}")
                        fap = flat[:]
                        fsrc = flat[:]
                        pair_dst = bass.AP(fap.tensor, fap.offset,
                                           [fap.ap[0], [3, R * N], [1, 2]])
                        off_sl = off[:, 2 * R * c: 2 * R * c + 2 * R]
                        nc.gpsimd.indirect_dma_start(
                            pair_dst, None, src,
                            IndirectOffsetOnAxis(ap=off_sl, axis=0))
                        if K_RING == "scalar":
                            nc.scalar.dma_start(gath[R * c: R * c + R, :], fsrc)
                        else:
                            nc.sync.dma_start(gath[R * c: R * c + R, :], fsrc)
                    dst = gath[:]
                    v0 = bass.AP(dst.tensor, dst.offset, [dst.ap[0], [3, N]])
                    v1 = bass.AP(dst.tensor, dst.offset + 1, [dst.ap[0], [3, N]])

                    if debug and t_idx == 0:
                        nc.sync.dma_start(dbg_pos[:, :], pos[:])
                        nc.sync.dma_start(dbg_off[:, :], off[:])
                        nc.sync.dma_start(dbg_gath[:, :], gath[:])
                        nc.sync.dma_start(dbg_w[:, :], w[:])

                    # out = v0 + w * v1
                    t1 = sb.tile([128, N], F32, tag="t1")
                    nc.vector.tensor_tensor(t1[:], w[:], v1, ALU.mult)
                    res = sb.tile([128, N], BF16, tag="res")
                    nc.vector.tensor_tensor(res[:], t1[:], v0, ALU.add)
                    nc.sync.dma_start(out_d[bi, h, r0:r0 + 128, :], res[:])
    nc.compile()
    return nc


_NC_CACHE = {}


def _make_runner(nc):
    """Cached jitted shard_map runner: traces once, keeps zero output
    buffers device-resident, skips donation (kernel writes every element)."""
    import jax
    from jax.sharding import Mesh, PartitionSpec, NamedSharding
    from jax.experimental.shard_map import shard_map
    from concourse import bass2jax

    bass2jax.install_neuronx_cc_hook()

    partition_name = nc.partition_id_tensor.name if nc.partition_id_tensor else None
    in_names, out_names, out_avals, zero_shapes = [], [], [], []
    for alloc in nc.m.functions[0].allocations:
        if not isinstance(alloc, mybir.MemoryLocationSet):
            continue
        name = alloc.memorylocations[0].name
        if alloc.kind == "ExternalInput":
            if name != partition_name:
                in_names.append(name)
        elif alloc.kind == "ExternalOutput":
            out_names.append(name)
            shape = tuple(alloc.tensor_shape)
            dtype = mybir.dt.np(alloc.dtype)
            out_avals.append(jax.core.ShapedArray(shape, dtype))
            zero_shapes.append((shape, dtype))
    n_params = len(in_names)
    all_in_names = list(in_names) + list(out_names)
    if partition_name is not None:
        all_in_names.append(partition_name)

    def _body(*args):
        operands = list(args)
        if partition_name is not None:
            operands.append(bass2jax.partition_id_tensor())
        outs = bass2jax._bass_exec_p.bind(
            *operands,
            out_avals=tuple(out_avals),
            in_names=tuple(all_in_names),
            out_names=tuple(out_names),
            lowering_input_output_aliases=(),
            sim_require_finite=True,
            sim_require_nnan=True,
            nc=nc,
        )
        return tuple(outs)

    devices = jax.devices()[:NCORES]
    mesh = Mesh(np.asarray(devices), ("core",))
    n_outs = len(out_names)
    in_specs = (PartitionSpec("core"),) * (n_params + n_outs)
    out_specs = (PartitionSpec("core"),) * n_outs
    jitted = jax.jit(
        shard_map(_body, mesh=mesh, in_specs=in_specs, out_specs=out_specs,
                  check_rep=False),
        keep_unused=True,
    )
    sharding = NamedSharding(mesh, PartitionSpec("core"))
    zeros_dev = [
        jax.device_put(np.zeros((NCORES * s[0], *s[1:]), d), sharding)
        for (s, d) in zero_shapes
    ]
    return jitted, zeros_dev, out_names


def kernel(query, attn_logits, pos_emb, Wh, Ww, npos_max):
    attn16 = np.asarray(attn_logits).astype(np.float16)
    q16 = np.asarray(query).astype(np.float16)
    p16 = np.asarray(pos_emb).astype(np.float16)

    if "nc" not in _NC_CACHE:
        _NC_CACHE["nc"] = _build(
            bacc.Bacc("TRN2", target_bir_lowering=False, num_devices=NCORES))
        _NC_CACHE["runner"] = _make_runner(_NC_CACHE["nc"])
    jitted, zeros_dev, out_names = _NC_CACHE["runner"]

    # inputs shard along axis 0 (8 cores x 8 b); P gets 8 stacked copies so
    # each core's shard is the full P
    outs = jitted(attn16, q16, np.tile(p16, (NCORES, 1)), *zeros_dev)
    out = np.asarray(outs[out_names.index("out")]).astype(np.float32)
    return out


# revision 22
# speedup vs baseline: 2.5086x; 1.0397x over previous
"""nn_CoPE2d Trainium2 Bass kernel. Self-contained.

Math per (b,h,half) tile (N=256, Wh=Ww=16, C=64, NPOS=288, 128 rows/tile):
  G = sigmoid(attn_logits[b,h,half])
  pos_h[n1,n2] = sum_{v'>=v, same u} G[u*16+v', n2]    (n1 = u*16+v)  -> PE matmul
  pos_w[n1,n2] = sum_{w'>=w2, same h2} G[n1, h2*16+w'] -> DVE reverse segmented scan
  pos = 16*pos_h + pos_w          (pos < 272, so the npos clamp never binds)
  L = Q @ P                        -> PE matmul  (logits_int)
  ff = floor(pos); w = pos - ff
  out = L[r,ff] + w*(L[r,ff+1] - L[r,ff])

Gather mechanism (walrus DynamicDMA facts, measured on hw by prior session):
  - descriptors are the dst AP's contiguous runs; one offset consumed per
    descriptor; offsets are consumed partition-fastest from the offset tile;
  - a 3-dim dst walks only dims[1:], so the dst is a single-partition flat
    buffer with [3,K],[1,2] runs (4B bf16 pair descriptors; hw scales the
    indirect address by the dst walk stride, so addr = 3*off elements);
  - the table is a stride-3 interleaved bf16 pair table TP2[:,3p]=L[p],
    TP2[:,3p+1]=L[p+1]-L[p] in DRAM (stride-2 gapless walks mis-gather on
    hw, and >4096 descriptors per indirect DMA crashes the device);
    gathered chunks are redistributed by one contiguous DMA per chunk;
    lerp = v0 + w*v1.
Sharding: data-parallel over B (8 b-values per core), ONE launch per core
with all 256 (b,h,half) tiles python-unrolled (walrus compile is fast).
The pair-table store and redistributes ride the ACT HWDGE ring so the Pool
SWDGE queue carries only the gathers (the bottleneck: ~13us gen + ~14us
SDMA transfer per tile; DVE ~4us, ACT ~3us, PE ~2us hide underneath).
"""
import sys
from contextlib import ExitStack

import numpy as np

sys.path.insert(0, "/opt/trn_rl_repo")

import concourse.bass as bass
import concourse.mybir as mybir
import concourse.tile as tile
from concourse import bacc
from concourse.bass import IndirectOffsetOnAxis
from concourse.bass_utils import run_bass_kernel_spmd

F32 = mybir.dt.float32
F16 = mybir.dt.float16
BF16 = mybir.dt.bfloat16
I32 = mybir.dt.int32
AF = mybir.ActivationFunctionType
ALU = mybir.AluOpType

B, NH, N, C, NPOS, SEG = 64, 16, 256, 64, 288, 16
NCORES = 8
BPC = B // NCORES  # b per core
R = 16             # rows per gather chunk (8 chunks per tile)

NCHUNK = 128 // R


def _m16_np():
    k = np.arange(128)
    m = np.arange(128)
    M = ((k[:, None] // SEG == m[None, :] // SEG)
         & (k[:, None] % SEG >= m[None, :] % SEG))
    return M.astype(np.float32)


def _maskr_np():
    t = np.arange(N)
    return np.broadcast_to((t % SEG != 0).astype(np.float32), (128, N)).copy()


def _rowbase_np():
    return (np.arange(128, dtype=np.int64) * NPOS).astype(np.float32).reshape(128, 1)


def _build(nc, bpc=BPC, debug=False):
    GDT = BF16
    A_d = nc.dram_tensor("A", [bpc, NH, N, N], F16, kind="ExternalInput")
    Q_d = nc.dram_tensor("Q", [bpc, NH, N, C], F16, kind="ExternalInput")
    P_d = nc.dram_tensor("P", [C, NPOS], F16, kind="ExternalInput")
    out_d = nc.dram_tensor("out", [bpc, NH, N, N], BF16, kind="ExternalOutput")
    if debug:
        dbg_pos = nc.dram_tensor("dbg_pos", [128, N], F32, kind="ExternalOutput")
        dbg_off = nc.dram_tensor("dbg_off", [128, N], I32, kind="ExternalOutput")
        dbg_gath = nc.dram_tensor("dbg_gath", [128, 3 * N], BF16,
                                  kind="ExternalOutput")
        dbg_w = nc.dram_tensor("dbg_w", [128, N], F32, kind="ExternalOutput")

    with tile.TileContext(nc) as tc, ExitStack() as ctx:
        sb = ctx.enter_context(tc.tile_pool(name="sb", bufs=2))
        flats = ctx.enter_context(tc.tile_pool(name="flats", bufs=1))
        const = ctx.enter_context(tc.tile_pool(name="const", bufs=1))
        psum = ctx.enter_context(tc.tile_pool(name="psum", bufs=2, space="PSUM"))
        psum2 = ctx.enter_context(tc.tile_pool(name="psum2", bufs=1, space="PSUM"))
        dra = ctx.enter_context(tc.tile_pool(name="dra", bufs=1, space="DRAM"))

        # constants
        M16_ld = const.tile([128, 128], F32, tag="M16ld")
        nc.sync.dma_start(M16_ld[:], nc.inline_tensor(_m16_np(), name="M16c")[:, :])
        M16 = const.tile([128, 128], F32, tag="M16")
        nc.vector.tensor_copy(M16[:], M16_ld[:])
        maskR = const.tile([128, N], F32, tag="maskR")
        nc.sync.dma_start(maskR[:], nc.inline_tensor(_maskr_np(), name="maskRc")[:, :])
        rowbase = const.tile([128, 1], F32, tag="rowbase")
        nc.sync.dma_start(rowbase[:], nc.inline_tensor(_rowbase_np(), name="rbc")[:, :])
        eye_ld = const.tile([128, 128], F32, tag="eyeld")
        nc.sync.dma_start(eye_ld[:], nc.inline_tensor(np.eye(128, dtype=np.float32),
                                                      name="eyec")[:, :])
        eye = const.tile([128, 128], F32, tag="eye")
        nc.vector.tensor_copy(eye[:], eye_ld[:])
        eye16_ld = const.tile([128, 128], F16, tag="eye16ld")
        nc.sync.dma_start(eye16_ld[:],
                          nc.inline_tensor(np.eye(128, dtype=np.float16),
                                           name="eye16c")[:, :])
        eye16 = const.tile([128, 128], F16, tag="eye16")
        nc.vector.tensor_copy(eye16[:], eye16_ld[:])
        P_ld = const.tile([C, NPOS], F16, tag="Pld")
        nc.sync.dma_start(P_ld[:], P_d[:, :])
        P_sb = const.tile([C, NPOS], F16, tag="P")
        nc.vector.tensor_copy(P_sb[:], P_ld[:])

        for bi in range(bpc):
            for h in range(NH):
                for half in range(2):
                    r0 = half * 128
                    t_idx = (bi * NH + h) * 2 + half

                    A_t = sb.tile([128, N], F16, tag="A")
                    nc.sync.dma_start(A_t[:], A_d[bi, h, r0:r0 + 128, :])
                    q_t = sb.tile([128, C], F16, tag="q")
                    nc.sync.dma_start(q_t[:], Q_d[bi, h, r0:r0 + 128, :])

                    G = sb.tile([128, N], F32, tag="G")
                    nc.scalar.activation(G[:], A_t[:], AF.Sigmoid)

                    # pos_h via masked-cumsum matmul, pos_w via DVE scan
                    psum_h = psum.tile([128, N], F32, tag="ph")
                    nc.tensor.matmul(psum_h[:], M16[:], G[:], start=True, stop=True)
                    ph_sb = sb.tile([128, N], F32, tag="ph_sb")
                    nc.scalar.copy(ph_sb[:], psum_h[:])
                    posw = sb.tile([128, N], F32, tag="pw")
                    nc.vector.tensor_tensor_scan(posw[:, ::-1], maskR[:], G[:, ::-1],
                                                 0.0, ALU.mult, ALU.add)
                    pos = sb.tile([128, N], F32, tag="pos")
                    nc.vector.scalar_tensor_tensor(pos[:], ph_sb[:], 16.0, posw[:],
                                                   ALU.mult, ALU.add)

                    # ff = floor(pos) robustly under either f32->i32 convert
                    # rounding mode (trunc or round-to-nearest)
                    fi = sb.tile([128, N], I32, tag="fi")
                    nc.vector.tensor_scalar(fi[:], pos[:], 0.0, None, ALU.add)
                    ff0 = sb.tile([128, N], F32, tag="ff0")
                    nc.vector.tensor_copy(ff0[:], fi[:])
                    gtm = sb.tile([128, N], F32, tag="gtm")
                    nc.vector.tensor_tensor(gtm[:], ff0[:], pos[:], ALU.is_gt)
                    ff = sb.tile([128, N], F32, tag="ff")
                    nc.vector.tensor_tensor(ff[:], ff0[:], gtm[:], ALU.subtract)
                    w = sb.tile([128, N], F32, tag="w")
                    nc.vector.scalar_tensor_tensor(w[:], ff[:], -1.0, pos[:],
                                                   ALU.mult, ALU.add)

                    # transpose q on device: qth[c, m] = q_t[m, c]
                    psum_qt = psum.tile([C, 128], F16, tag="pqt")
                    nc.tensor.transpose(psum_qt[:], q_t[:], eye16[:])
                    qth = sb.tile([C, 128], F16, tag="qth")
                    nc.scalar.copy(qth[:], psum_qt[:])

                    psum_L = psum.tile([128, NPOS], F32, tag="pl")
                    nc.tensor.matmul(psum_L[:], qth[:], P_sb[:], start=True,
                                     stop=True)
                    Lsb = sb.tile([128, NPOS], F32, tag="Lsb")
                    nc.scalar.copy(Lsb[:], psum_L[:])

                    # stride-3 pair table (the stride-2 gapless layout mis-
                    # gathers on hw): TP2[:,3p] = L[p], TP2[:,3p+1] = D[p]
                    TP2 = sb.tile([128, 3 * NPOS], GDT, tag="TP2")
                    t0 = TP2[:]
                    nc.vector.tensor_copy(
                        bass.AP(t0.tensor, t0.offset, [t0.ap[0], [3, NPOS]]),
                        Lsb[:, 0:NPOS])
                    nc.vector.scalar_tensor_tensor(
                        bass.AP(t0.tensor, t0.offset + 1, [t0.ap[0], [3, NPOS - 1]]),
                        Lsb[:, 0:NPOS - 1], -1.0, Lsb[:, 1:NPOS],
                        ALU.mult, ALU.add)
                    # unread lanes (3p+2 and the last D slot); init for sim
                    nc.vector.memset(
                        bass.AP(t0.tensor, t0.offset + 2, [t0.ap[0], [3, NPOS]]), 0)
                    nc.vector.memset(
                        bass.AP(t0.tensor, t0.offset + 3 * (NPOS - 1) + 1,
                                [t0.ap[0], [1, 1]]), 0)
                    TD = dra.tile([128, 3 * NPOS], GDT, tag="TD")
                    td_ap = TD[:]
                    assert td_ap.offset == 0, "pair table must sit at offset 0"
                    nc.scalar.dma_start(td_ap, TP2[:])

                    # transposed offsets: off[q, 2a+b] = ff[a, b*128+q] + a*NPOS
                    offf = sb.tile([128, N], F32, tag="offf")
                    nc.vector.tensor_scalar(offf[:], ff[:], rowbase[:], None, ALU.add)
                    ptA = psum2.tile([128, 128], F32, tag="tA")
                    nc.tensor.transpose(ptA[:], offf[:, 0:128], eye[:])
                    ptB = psum2.tile([128, 128], F32, tag="tB")
                    nc.tensor.transpose(ptB[:], offf[:, 128:256], eye[:])
                    off = sb.tile([128, N], I32, tag="off")
                    oap = off[:]
                    nc.vector.tensor_copy(
                        bass.AP(oap.tensor, oap.offset, [oap.ap[0], [2, 128]]),
                        ptA[:])
                    nc.vector.tensor_copy(
                        bass.AP(oap.tensor, oap.offset + 1, [oap.ap[0], [2, 128]]),
                        ptB[:])

                    # pair-gather: 2 chunks of R=64 rows; src AP shaped as
                    # pairs so sim coef(=2) matches hw dst-walk-stride scaling
                    gath = sb.tile([128, 3 * N], GDT, tag="gath")
                    src = bass.AP(td_ap.tensor, 0, [[3, 128 * NPOS], [1, 3]])
                    for c in range(NCHUNK):
                        flat = flats.tile([1, 3 * R * N], GDT,
                                          tag=f"flat{c % 2}")
                        fap = flat[:]
                        fsrc = flat[:]
                        pair_dst = bass.AP(fap.tensor, fap.offset,
                                           [fap.ap[0], [3, R * N], [1, 2]])
                        off_sl = off[:, 2 * R * c: 2 * R * c + 2 * R]
                        nc.gpsimd.indirect_dma_start(
                            pair_dst, None, src,
                            IndirectOffsetOnAxis(ap=off_sl, axis=0))
                        nc.scalar.dma_start(gath[R * c: R * c + R, :],
                                            fsrc)
                    dst = gath[:]
                    v0 = bass.AP(dst.tensor, dst.offset, [dst.ap[0], [3, N]])
                    v1 = bass.AP(dst.tensor, dst.offset + 1, [dst.ap[0], [3, N]])

                    if debug and t_idx == 0:
                        nc.sync.dma_start(dbg_pos[:, :], pos[:])
                        nc.sync.dma_start(dbg_off[:, :], off[:])
                        nc.sync.dma_start(dbg_gath[:, :], gath[:])
                        nc.sync.dma_start(dbg_w[:, :], w[:])

                    # out = v0 + w * v1
                    t1 = sb.tile([128, N], F32, tag="t1")
                    nc.vector.tensor_tensor(t1[:], w[:], v1, ALU.mult)
                    res = sb.tile([128, N], BF16, tag="res")
                    nc.vector.tensor_tensor(res[:], t1[:], v0, ALU.add)
                    nc.sync.dma_start(out_d[bi, h, r0:r0 + 128, :], res[:])
    nc.compile()
    return nc


_NC_CACHE = {}


def _make_runner(nc):
    """Cached jitted shard_map runner: traces once, keeps zero output
    buffers device-resident, skips donation (kernel writes every element)."""
    import jax
    from jax.sharding import Mesh, PartitionSpec, NamedSharding
    from jax.experimental.shard_map import shard_map
    from concourse import bass2jax

    bass2jax.install_neuronx_cc_hook()

    partition_name = nc.partition_id_tensor.name if nc.partition_id_tensor else None
    in_names, out_names, out_avals, zero_shapes = [], [], [], []
    for alloc in nc.m.functions[0].allocations:
        if not isinstance(alloc, mybir.MemoryLocationSet):
            continue
        name = alloc.memorylocations[0].name
        if alloc.kind == "ExternalInput":
            if name != partition_name:
                in_names.append(name)
        elif alloc.kind == "ExternalOutput":
            out_names.append(name)
            shape = tuple(alloc.tensor_shape)
            dtype = mybir.dt.np(alloc.dtype)
            out_avals.append(jax.core.ShapedArray(shape, dtype))
            zero_shapes.append((shape, dtype))
    n_params = len(in_names)
    all_in_names = list(in_names) + list(out_names)
    if partition_name is not None:
        all_in_names.append(partition_name)

    def _body(*args):
        operands = list(args)
        if partition_name is not None:
            operands.append(bass2jax.partition_id_tensor())
        outs = bass2jax._bass_exec_p.bind(
            *operands,
            out_avals=tuple(out_avals),
            in_names=tuple(all_in_names),
            out_names=tuple(out_names),
            lowering_input_output_aliases=(),
            sim_require_finite=True,
            sim_require_nnan=True,
            nc=nc,
        )
        return tuple(outs)

    devices = jax.devices()[:NCORES]
    mesh = Mesh(np.asarray(devices), ("core",))
    n_outs = len(out_names)
    in_specs = (PartitionSpec("core"),) * (n_params + n_outs)
    out_specs = (PartitionSpec("core"),) * n_outs
    jitted = jax.jit(
        shard_map(_body, mesh=mesh, in_specs=in_specs, out_specs=out_specs,
                  check_rep=False),
        keep_unused=True,
    )
    sharding = NamedSharding(mesh, PartitionSpec("core"))
    zeros_dev = [
        jax.device_put(np.zeros((NCORES * s[0], *s[1:]), d), sharding)
        for (s, d) in zero_shapes
    ]
    return jitted, zeros_dev, out_names


def kernel(query, attn_logits, pos_emb, Wh, Ww, npos_max):
    attn16 = np.asarray(attn_logits).astype(np.float16)
    q16 = np.asarray(query).astype(np.float16)
    p16 = np.asarray(pos_emb).astype(np.float16)

    if "nc" not in _NC_CACHE:
        _NC_CACHE["nc"] = _build(
            bacc.Bacc("TRN2", target_bir_lowering=False, num_devices=NCORES))
        _NC_CACHE["runner"] = _make_runner(_NC_CACHE["nc"])
    jitted, zeros_dev, out_names = _NC_CACHE["runner"]

    # inputs shard along axis 0 (8 cores x 8 b); P gets 8 stacked copies so
    # each core's shard is the full P
    outs = jitted(attn16, q16, np.tile(p16, (NCORES, 1)), *zeros_dev)
    out = np.asarray(outs[out_names.index("out")]).astype(np.float32)
    return out
